# revision 2
# baseline (speedup 1.0000x reference)
"""CPD reconstruction v5: dma_gather + joint residue-class slot ordering.

Per core (125k entries), all on-device work in one Bass program:

- Host sorts entries by joint class g = (i0%4)*16 + (i1%4)*4 + (i2%4)
  (64 groups), pads each group to whole 128-slot columns; slot = col*128+p.
  Group column counts are maxed over the 8 cores so one SPMD program fits
  all; per-core group contents differ only in the (host-built) idx arrays.
- Processing in PASSES over group ranges.  Per pass, per mode m, per class
  c: the idx stream = concat of groups g in the pass with c_m(g) == c, each
  padded to whole columns; dma_gather calls chunk the stream (<=15872 idx).
  idx value = row >> 2 (< 25000, int16-safe); the mode's table is viewed
  [25000, 128] fp16 and the in-AP is column-shifted by c*32 so each
  256B-strided descriptor reads exactly the wanted 64B row.
- Gathers land in list order = pass-local stream order; the product runs
  per GROUP with host-computed per-mode tile offsets (contiguous slices),
  2 fp16 muls + rank reduce_sum into the slot-ordered f32 output.
- Host maps the padded slot grid back to entry order (free).
"""

import numpy as np

NNZ = 1_000_000
RANK = 32
ROWS = 100_000
N_CORES = 8
N_PER_CORE = NNZ // N_CORES  # 125_000
P = 128
MAXCOLS = 124  # per-call column cap (15872 idx < Q7 scratch limit)
N_PASS = 2

_cache: dict = {}


def dma_gather_raw(gp, out_ap, in_ap, idxs_ap, num_idxs, elem_size,
                   elem_step=None, queue_num=0):
    """bass dma_gather minus the 256B elem_size assert (transpose-only)."""
    import concourse.mybir as mybir
    from concourse.bass import exact_div
    import concourse.ap_utils as ap_utils

    assert idxs_ap.dtype == mybir.dt.int16
    assert in_ap.dtype == out_ap.dtype
    if elem_step is None:
        elem_step = elem_size
    assert ap_utils.ap_is_contiguous(in_ap.ap[1:])
    assert ap_utils.ap_is_contiguous(out_ap.ap[1:])
    assert ap_utils.ap_is_contiguous(idxs_ap.ap[1:])
    assert in_ap.ap[-1][1] == elem_size
    assert out_ap.ap[-1][1] == elem_size
    assert in_ap.ap[0][0] == elem_step
    assert out_ap.ap[0][1] * out_ap.ap[1][1] == -(-num_idxs // 128) * 128
    stride_bytes = elem_step * mybir.dt.size(in_ap.dtype)
    stride_bytes_256 = exact_div(stride_bytes, 256)
    assert stride_bytes_256 < 256
    _in_ap = gp.lower_ap_dma(in_ap, for_custom_bir_dma=True)
    _idxs_ap = gp.lower_ap(idxs_ap)
    _out_ap = gp.lower_ap(out_ap)
    return gp.add_instruction(
        mybir.InstDMAGatherAnt(
            name=gp.bass.get_next_instruction_name(),
            ins=[*_in_ap, _idxs_ap, gp.lower_val_access(gp.to_reg(num_idxs))],
            outs=[_out_ap],
            transpose=False,
            num_idxs=num_idxs,
            elem_size=elem_size,
            stride_bytes_256=stride_bytes_256,
            gen_mode=0,
            single_packet=False,
            queue_num=queue_num,
            sbuf_tokens_per_rank=0,
            sbuf_free_dim_per_rank=0,
            sbuf_free_dim_pad_per_rank=0,
            sbuf_byte_offset=0,
        )
    )


def _split_multi_waits(nc, mybir):
    for blk in nc.m.functions[0].blocks:
        new_insts = []
        for inst in blk.instructions:
            si = inst.sync_info
            if si is not None and si.on_wait and len(si.on_wait) > 1:
                extra, keep = list(si.on_wait[:-1]), [si.on_wait[-1]]
                for j, w in enumerate(extra):
                    new_insts.append(
                        mybir.InstEventSemaphore(
                            name=f"{inst.name}-esw{j}",
                            engine=inst.engine,
                            ins=[],
                            outs=[],
                            sync_info=mybir.SyncInfo(on_wait=[w], on_update=[]),
                        )
                    )
                si.on_wait = keep
            new_insts.append(inst)
        blk.instructions = new_insts


QM = np.stack(
    [(np.arange(64) >> 4) & 3, (np.arange(64) >> 2) & 3, np.arange(64) & 3]
)  # [3, 64]: class of group g per mode


def make_layout(ncols: np.ndarray):
    """Shared (all-core) layout from padded per-group column counts.

    Returns dict with:
      T            total slot columns
      gcol[65]     slot-grid column offset of each group
      passes       list of (ga, gb) group ranges
      streams      per pass, per mode: list of (class, [groups], stream_cols)
      g_tile_off   [n_pass][3][64] tile column offset of each group
      calls        per pass: list of (mode, class, tile_col0, tile_col1,
                   idx_seg_off) with idx_seg_off = column offset into that
                   pass+mode idx segment
      pass_cols    per pass: total columns (same for each mode)
      idx_layout   per pass, per mode: ordered [groups] defining idx order
    """
    gcol = np.concatenate([[0], np.cumsum(ncols)]).astype(np.int64)
    T = int(gcol[-1])
    # split into N_PASS group ranges with ~equal columns
    bounds = [0]
    for k in range(1, N_PASS):
        target = T * k // N_PASS
        bounds.append(int(np.searchsorted(gcol, target)))
    bounds.append(64)
    passes = [(bounds[i], bounds[i + 1]) for i in range(N_PASS)]

    g_tile_off = np.full((N_PASS, 3, 64), -1, dtype=np.int64)
    calls = []
    pass_cols = []
    idx_groups = []
    for pi, (ga, gb) in enumerate(passes):
        pcalls = []
        pgroups = [[], [], []]
        for m in range(3):
            off = 0
            for c in range(4):
                groups = [g for g in range(ga, gb) if QM[m, g] == c and ncols[g] > 0]
                if not groups:
                    continue
                c0 = off
                for g in groups:
                    g_tile_off[pi, m, g] = off
                    off += int(ncols[g])
                    pgroups[m].append(g)
                # chunk [c0, off) into calls
                a = c0
                while a < off:
                    b = min(a + MAXCOLS, off)
                    pcalls.append((m, c, a, b))
                    a = b
            if m == 0:
                pc = off
            else:
                assert off == pc
        calls.append(pcalls)
        pass_cols.append(pc)
        idx_groups.append(pgroups)
    return {
        "T": T,
        "gcol": gcol,
        "passes": passes,
        "g_tile_off": g_tile_off,
        "calls": calls,
        "pass_cols": pass_cols,
        "idx_groups": idx_groups,
    }


def _build(ncols_key):
    import concourse.bass as bass
    import concourse.mybir as mybir
    from concourse.tile import TileContext
    from concourse import library_config
    from concourse.library_overlay import lower_extended_insts

    ncols = np.array(ncols_key, dtype=np.int64)
    L = make_layout(ncols)
    T = L["T"]
    maxpc = max(L["pass_cols"])
    S16_per_pass_mode = [pc * 8 for pc in L["pass_cols"]]  # int16 cols per P
    # idx dram layout: per pass, per mode, stream of pc*128 idx wrapped
    tot_s16 = 3 * sum(S16_per_pass_mode)

    nc = bass.Bass(num_swdge_queues=4)
    ftabs = [
        nc.dram_tensor(f"f{m}p", [ROWS // 4, 128], mybir.dt.float16,
                       kind="ExternalInput")
        for m in range(3)
    ]
    idx_d = nc.dram_tensor("idx16", [P, tot_s16], mybir.dt.int16,
                           kind="ExternalInput")
    out_d = nc.dram_tensor("out", [P, T], mybir.dt.float32, kind="ExternalOutput")

    with TileContext(nc) as tc:
        nc.gpsimd.load_library(library_config.attnmlp)
        with (
            tc.tile_pool(name="io", bufs=1) as io_pool,
            tc.tile_pool(name="gat", bufs=1) as gat_pool,
            tc.tile_pool(name="prd", bufs=2) as prd_pool,
        ):
            out_sb = io_pool.tile([P, T], mybir.dt.float32)
            idx_all = io_pool.tile([P, tot_s16], mybir.dt.int16)
            nc.sync.dma_start(out=idx_all[:], in_=idx_d[:])
            g_tiles = []
            for m in range(3):
                gt = gat_pool.tile(
                    [P, maxpc * RANK], mybir.dt.float16, tag=f"g{m}",
                    name=f"g{m}",
                )
                g_tiles.append(gt)
            qrot = 0
            seg_base = 0
            for pi, (ga, gb) in enumerate(L["passes"]):
                pc = L["pass_cols"][pi]
                for (m, c, a, b) in L["calls"][pi]:
                    nidx = (b - a) * P
                    dma_gather_raw(
                        nc.gpsimd,
                        out_ap=g_tiles[m][:, a * RANK : b * RANK].rearrange(
                            "p (s r) -> p s r", r=RANK
                        ),
                        in_ap=ftabs[m][:, c * RANK : (c + 1) * RANK],
                        idxs_ap=idx_all[
                            :,
                            seg_base + m * pc * 8 + a * 8 : seg_base
                            + m * pc * 8
                            + b * 8,
                        ],
                        num_idxs=nidx,
                        elem_size=RANK,
                        elem_step=128,
                        queue_num=qrot % 4,
                    )
                    qrot += 1
                seg_base += 3 * pc * 8
                # products per group
                for g in range(ga, gb):
                    n_g = int(ncols[g])
                    if n_g == 0:
                        continue
                    outc = int(L["gcol"][g])
                    v = [
                        g_tiles[m][
                            :,
                            int(L["g_tile_off"][pi, m, g])
                            * RANK : (int(L["g_tile_off"][pi, m, g]) + n_g)
                            * RANK,
                        ].rearrange("p (s r) -> p s r", r=RANK)
                        for m in range(3)
                    ]
                    tmp = prd_pool.tile([P, 20 * RANK], mybir.dt.float16, tag="t")
                    j0 = 0
                    while j0 < n_g:
                        C = min(20, n_g - j0)
                        tv = tmp[:, : C * RANK].rearrange(
                            "p (s r) -> p s r", r=RANK
                        )
                        nc.vector.tensor_mul(
                            out=tv,
                            in0=v[0][:, j0 : j0 + C],
                            in1=v[1][:, j0 : j0 + C],
                        )
                        nc.vector.tensor_mul(out=tv, in0=tv, in1=v[2][:, j0 : j0 + C])
                        nc.vector.reduce_sum(
                            out=out_sb[:, outc + j0 : outc + j0 + C],
                            in_=tv,
                            axis=mybir.AxisListType.X,
                        )
                        j0 += C
            nc.sync.dma_start(out=out_d[:], in_=out_sb[:])

    _fix_gather_queues(nc, mybir)
    lower_extended_insts(nc)
    _split_multi_waits(nc, mybir)
    return nc, L


def _fix_gather_queues(nc, mybir):
    """Align each gather's SWDGE queue with its Tile-assigned DMASW sem lane
    (a sem lane must only ever be updated from one queue)."""
    import re

    for blk in nc.m.functions[0].blocks:
        for inst in blk.instructions:
            if isinstance(inst, mybir.InstDMAGatherAnt):
                si = inst.sync_info
                lane = None
                for u in list(getattr(si, "on_update", []) or []):
                    m = re.search(r"DMASW(\d+)", repr(u))
                    if m:
                        lane = int(m.group(1))
                        break
                assert lane is not None, f"no DMASW sem on {inst.name}"
                inst.queue_num = lane % 4


def wrap_idxs(idx: np.ndarray) -> np.ndarray:
    """[N] -> [128, N/16] int16: idx i at [i%16, i//16], replicated x8."""
    n = len(idx)
    assert n % 16 == 0
    w = idx.reshape(n // 16, 16).T.astype(np.int16)
    return np.tile(w, (8, 1))


def _prep(idxs, f0, f1, f2):
    idxs = np.asarray(idxs)
    idx3_full = idxs.astype(np.int32) if idxs.dtype != np.int32 else idxs
    ftabs = [
        np.ascontiguousarray(
            np.asarray(f, dtype=np.float32).astype(np.float16).reshape(ROWS // 4, 128)
        )
        for f in (f0, f1, f2)
    ]

    cores = []
    ncols = np.zeros(64, dtype=np.int64)
    for cidx in range(N_CORES):
        sl = idx3_full[cidx * N_PER_CORE : (cidx + 1) * N_PER_CORE]
        cls = sl & 3
        joint = (cls[:, 0] << 4) | (cls[:, 1] << 2) | cls[:, 2]
        order = np.argsort(joint, kind="stable")
        cnt = np.bincount(joint, minlength=64)
        cores.append((sl, order, cnt))
        ncols = np.maximum(ncols, -(-cnt // P))

    ncols_key = tuple(int(x) for x in ncols)
    if ncols_key not in _cache:
        _cache[ncols_key] = _build(ncols_key)
    nc, L = _cache[ncols_key]

    in_maps = []
    entry_pos_all = []
    for sl, order, cnt in cores:
        gstart = np.concatenate([[0], np.cumsum(cnt)])
        n = sl.shape[0]
        # entry -> padded slot position (slot grid: group-major, col*128+p)
        entry_pos = np.empty(n, dtype=np.int64)
        pos_of_sorted = (
            np.repeat(L["gcol"][:-1] * P, cnt)
            + np.arange(n)
            - np.repeat(gstart[:-1], cnt)
        )
        entry_pos[order] = pos_of_sorted
        entry_pos_all.append(entry_pos)

        segs = []
        for pi in range(N_PASS):
            pc = L["pass_cols"][pi]
            for m in range(3):
                stream = np.zeros(pc * P, dtype=np.int16)
                rows_m = sl[order, m]  # slot order
                for g in L["idx_groups"][pi][m]:
                    s0, s1 = gstart[g], gstart[g + 1]
                    dst = int(L["g_tile_off"][pi, m, g]) * P
                    stream[dst : dst + (s1 - s0)] = (rows_m[s0:s1] >> 2).astype(
                        np.int16
                    )
                segs.append(wrap_idxs(stream))
        idx_arr = np.ascontiguousarray(np.concatenate(segs, axis=1))
        in_maps.append(
            {"idx16": idx_arr, "f0p": ftabs[0], "f1p": ftabs[1], "f2p": ftabs[2]}
        )
    return nc, L, in_maps, entry_pos_all


def run(inputs: dict, trace: bool = False):
    from concourse.bass_utils import run_bass_kernel_spmd

    nc, L, in_maps, entry_pos_all = _prep(
        inputs["idxs"], inputs["f0"], inputs["f1"], inputs["f2"]
    )
    res = run_bass_kernel_spmd(
        nc, in_maps, core_ids=list(range(N_CORES)), trace=trace
    )
    outs = []
    for cidx in range(N_CORES):
        flat = res.results[cidx]["out"]  # [P, T] f32; pos = col*128 + p
        vals = np.ascontiguousarray(flat.T).reshape(-1)  # pos-major
        outs.append(vals[entry_pos_all[cidx]])
    return np.concatenate(outs), res


def kernel(**inputs) -> np.ndarray:
    out, _ = run(inputs, trace=False)
    return out


# revision 4
# speedup vs baseline: 1.6666x; 1.6666x over previous
"""CPD reconstruction v5: dma_gather + joint residue-class slot ordering.

Per core (125k entries), all on-device work in one Bass program:

- Host sorts entries by joint class g = (i0%4)*16 + (i1%4)*4 + (i2%4)
  (64 groups), pads each group to whole 128-slot columns; slot = col*128+p.
  Group column counts are maxed over the 8 cores so one SPMD program fits
  all; per-core group contents differ only in the (host-built) idx arrays.
- Processing in PASSES over group ranges.  Per pass, per mode m, per class
  c: the idx stream = concat of groups g in the pass with c_m(g) == c, each
  padded to whole columns; dma_gather calls chunk the stream (<=15872 idx).
  idx value = row >> 2 (< 25000, int16-safe); the mode's table is viewed
  [25000, 128] fp16 and the in-AP is column-shifted by c*32 so each
  256B-strided descriptor reads exactly the wanted 64B row.
- Gathers land in list order = pass-local stream order; the product runs
  per GROUP with host-computed per-mode tile offsets (contiguous slices),
  2 fp16 muls + rank reduce_sum into the slot-ordered f32 output.
- Host maps the padded slot grid back to entry order (free).
"""

import numpy as np

NNZ = 1_000_000
RANK = 32
ROWS = 100_000
N_CORES = 8
N_PER_CORE = NNZ // N_CORES  # 125_000
P = 128
MAXCOLS = 62  # per-call column cap
N_PASS = 2

_cache: dict = {}


_reg_cache: dict = {}


def dma_gather_raw(gp, out_ap, in_ap, idxs_ap, num_idxs, elem_size,
                   elem_step=None, queue_num=0):
    """bass dma_gather minus the 256B elem_size assert (transpose-only)."""
    import concourse.mybir as mybir
    from concourse.bass import exact_div
    import concourse.ap_utils as ap_utils

    assert idxs_ap.dtype == mybir.dt.int16
    assert in_ap.dtype == out_ap.dtype
    if elem_step is None:
        elem_step = elem_size
    assert ap_utils.ap_is_contiguous(in_ap.ap[1:])
    assert ap_utils.ap_is_contiguous(out_ap.ap[1:])
    assert ap_utils.ap_is_contiguous(idxs_ap.ap[1:])
    assert in_ap.ap[-1][1] == elem_size
    assert out_ap.ap[-1][1] == elem_size
    assert in_ap.ap[0][0] == elem_step
    assert out_ap.ap[0][1] * out_ap.ap[1][1] == -(-num_idxs // 128) * 128
    stride_bytes = elem_step * mybir.dt.size(in_ap.dtype)
    stride_bytes_256 = exact_div(stride_bytes, 256)
    assert stride_bytes_256 < 256
    _in_ap = gp.lower_ap_dma(in_ap, for_custom_bir_dma=True)
    _idxs_ap = gp.lower_ap(idxs_ap)
    _out_ap = gp.lower_ap(out_ap)
    return gp.add_instruction(
        mybir.InstDMAGatherAnt(
            name=gp.bass.get_next_instruction_name(),
            ins=[*_in_ap, _idxs_ap, _num_idxs_reg(gp, num_idxs)],
            outs=[_out_ap],
            transpose=False,
            num_idxs=num_idxs,
            elem_size=elem_size,
            stride_bytes_256=stride_bytes_256,
            gen_mode=0,
            single_packet=False,
            queue_num=queue_num,
            sbuf_tokens_per_rank=0,
            sbuf_free_dim_per_rank=0,
            sbuf_free_dim_pad_per_rank=0,
            sbuf_byte_offset=0,
        )
    )


def _num_idxs_reg(gp, num_idxs):
    key = (id(gp.bass), num_idxs)
    if key not in _reg_cache:
        _reg_cache[key] = gp.lower_val_access(gp.to_reg(num_idxs))
    return _reg_cache[key]


def _split_multi_waits(nc, mybir):
    for blk in nc.m.functions[0].blocks:
        new_insts = []
        for inst in blk.instructions:
            si = inst.sync_info
            if si is not None and si.on_wait and len(si.on_wait) > 1:
                extra, keep = list(si.on_wait[:-1]), [si.on_wait[-1]]
                for j, w in enumerate(extra):
                    new_insts.append(
                        mybir.InstEventSemaphore(
                            name=f"{inst.name}-esw{j}",
                            engine=inst.engine,
                            ins=[],
                            outs=[],
                            sync_info=mybir.SyncInfo(on_wait=[w], on_update=[]),
                        )
                    )
                si.on_wait = keep
            new_insts.append(inst)
        blk.instructions = new_insts


QM = np.stack(
    [(np.arange(64) >> 4) & 3, (np.arange(64) >> 2) & 3, np.arange(64) & 3]
)  # [3, 64]: class of group g per mode


def make_layout(ncols: np.ndarray):
    """Shared (all-core) layout from padded per-group column counts.

    Returns dict with:
      T            total slot columns
      gcol[65]     slot-grid column offset of each group
      passes       list of (ga, gb) group ranges
      streams      per pass, per mode: list of (class, [groups], stream_cols)
      g_tile_off   [n_pass][3][64] tile column offset of each group
      calls        per pass: list of (mode, class, tile_col0, tile_col1,
                   idx_seg_off) with idx_seg_off = column offset into that
                   pass+mode idx segment
      pass_cols    per pass: total columns (same for each mode)
      idx_layout   per pass, per mode: ordered [groups] defining idx order
    """
    gcol = np.concatenate([[0], np.cumsum(ncols)]).astype(np.int64)
    T = int(gcol[-1])
    # split into N_PASS group ranges with ~equal columns
    bounds = [0]
    for k in range(1, N_PASS):
        target = T * k // N_PASS
        bounds.append(int(np.searchsorted(gcol, target)))
    bounds.append(64)
    passes = [(bounds[i], bounds[i + 1]) for i in range(N_PASS)]

    g_tile_off = np.full((N_PASS, 3, 64), -1, dtype=np.int64)
    calls = []
    pass_cols = []
    idx_groups = []
    for pi, (ga, gb) in enumerate(passes):
        pcalls = []
        pgroups = [[], [], []]
        for m in range(3):
            off = 0
            for c in range(4):
                groups = [g for g in range(ga, gb) if QM[m, g] == c and ncols[g] > 0]
                if not groups:
                    continue
                c0 = off
                for g in groups:
                    g_tile_off[pi, m, g] = off
                    off += int(ncols[g])
                    pgroups[m].append(g)
                # chunk [c0, off) into calls
                a = c0
                while a < off:
                    b = min(a + MAXCOLS, off)
                    pcalls.append((m, c, a, b))
                    a = b
            if m == 0:
                pc = off
            else:
                assert off == pc
        calls.append(pcalls)
        pass_cols.append(pc)
        idx_groups.append(pgroups)
    return {
        "T": T,
        "gcol": gcol,
        "passes": passes,
        "g_tile_off": g_tile_off,
        "calls": calls,
        "pass_cols": pass_cols,
        "idx_groups": idx_groups,
    }


def _build(ncols_key):
    import concourse.bass as bass
    import concourse.mybir as mybir
    from concourse.tile import TileContext
    from concourse import library_config
    from concourse.library_overlay import lower_extended_insts

    ncols = np.array(ncols_key, dtype=np.int64)
    L = make_layout(ncols)
    T = L["T"]
    maxpc = max(L["pass_cols"])
    S16_per_pass_mode = [pc * 8 for pc in L["pass_cols"]]  # int16 cols per P
    # idx dram layout: per pass, per mode, stream of pc*128 idx wrapped
    tot_s16 = 3 * sum(S16_per_pass_mode)

    nc = bass.Bass(num_swdge_queues=4)
    ftabs = [
        nc.dram_tensor(f"f{m}p", [ROWS // 4, 128], mybir.dt.float16,
                       kind="ExternalInput")
        for m in range(3)
    ]
    idx_d = nc.dram_tensor("idx16", [P, tot_s16], mybir.dt.int16,
                           kind="ExternalInput")
    out_d = nc.dram_tensor("out", [P, T], mybir.dt.float32, kind="ExternalOutput")

    with TileContext(nc) as tc:
        nc.gpsimd.load_library(library_config.attnmlp)
        with (
            tc.tile_pool(name="io", bufs=1) as io_pool,
            tc.tile_pool(name="gat", bufs=1) as gat_pool,
            tc.tile_pool(name="prd", bufs=2) as prd_pool,
        ):
            out_sb = io_pool.tile([P, T], mybir.dt.float32)
            idx_all = io_pool.tile([P, tot_s16], mybir.dt.int16)
            nc.sync.dma_start(out=idx_all[:], in_=idx_d[:])
            g_tiles = []
            for m in range(3):
                gt = gat_pool.tile(
                    [P, maxpc * RANK], mybir.dt.float16, tag=f"g{m}",
                    name=f"g{m}",
                )
                g_tiles.append(gt)
            qrot = 0
            seg_base = 0
            for pi, (ga, gb) in enumerate(L["passes"]):
                pc = L["pass_cols"][pi]
                for (m, c, a, b) in L["calls"][pi]:
                    nidx = (b - a) * P
                    dma_gather_raw(
                        nc.gpsimd,
                        out_ap=g_tiles[m][:, a * RANK : b * RANK].rearrange(
                            "p (s r) -> p s r", r=RANK
                        ),
                        in_ap=ftabs[m][:, c * RANK : (c + 1) * RANK],
                        idxs_ap=idx_all[
                            :,
                            seg_base + m * pc * 8 + a * 8 : seg_base
                            + m * pc * 8
                            + b * 8,
                        ],
                        num_idxs=nidx,
                        elem_size=RANK,
                        elem_step=128,
                        queue_num=qrot % 4,
                    )
                    qrot += 1
                seg_base += 3 * pc * 8
                # products per group
                for g in range(ga, gb):
                    n_g = int(ncols[g])
                    if n_g == 0:
                        continue
                    outc = int(L["gcol"][g])
                    v = [
                        g_tiles[m][
                            :,
                            int(L["g_tile_off"][pi, m, g])
                            * RANK : (int(L["g_tile_off"][pi, m, g]) + n_g)
                            * RANK,
                        ].rearrange("p (s r) -> p s r", r=RANK)
                        for m in range(3)
                    ]
                    tmp = prd_pool.tile([P, 20 * RANK], mybir.dt.float16, tag="t")
                    j0 = 0
                    while j0 < n_g:
                        C = min(20, n_g - j0)
                        tv = tmp[:, : C * RANK].rearrange(
                            "p (s r) -> p s r", r=RANK
                        )
                        nc.vector.tensor_mul(
                            out=tv,
                            in0=v[0][:, j0 : j0 + C],
                            in1=v[1][:, j0 : j0 + C],
                        )
                        nc.vector.tensor_mul(out=tv, in0=tv, in1=v[2][:, j0 : j0 + C])
                        nc.vector.reduce_sum(
                            out=out_sb[:, outc + j0 : outc + j0 + C],
                            in_=tv,
                            axis=mybir.AxisListType.X,
                        )
                        j0 += C
            nc.sync.dma_start(out=out_d[:], in_=out_sb[:])

    _fix_gather_queues(nc, mybir)
    lower_extended_insts(nc)
    _split_multi_waits(nc, mybir)
    return nc, L


def _fix_gather_queues(nc, mybir):
    """Align each gather's SWDGE queue with its Tile-assigned DMASW sem lane
    (a sem lane must only ever be updated from one queue)."""
    import re

    for blk in nc.m.functions[0].blocks:
        for inst in blk.instructions:
            if isinstance(inst, mybir.InstDMAGatherAnt):
                si = inst.sync_info
                lane = None
                for u in list(getattr(si, "on_update", []) or []):
                    m = re.search(r"DMASW(\d+)", repr(u))
                    if m:
                        lane = int(m.group(1))
                        break
                assert lane is not None, f"no DMASW sem on {inst.name}"
                inst.queue_num = lane % 4


def wrap_idxs(idx: np.ndarray) -> np.ndarray:
    """[N] -> [128, N/16] int16: idx i at [i%16, i//16], replicated x8."""
    n = len(idx)
    assert n % 16 == 0
    w = idx.reshape(n // 16, 16).T.astype(np.int16)
    return np.tile(w, (8, 1))


def _prep(idxs, f0, f1, f2):
    idxs = np.asarray(idxs)
    idx3_full = idxs.astype(np.int32) if idxs.dtype != np.int32 else idxs
    ftabs = [
        np.ascontiguousarray(
            np.asarray(f, dtype=np.float32).astype(np.float16).reshape(ROWS // 4, 128)
        )
        for f in (f0, f1, f2)
    ]

    cores = []
    ncols = np.zeros(64, dtype=np.int64)
    for cidx in range(N_CORES):
        sl = idx3_full[cidx * N_PER_CORE : (cidx + 1) * N_PER_CORE]
        cls = sl & 3
        joint = (cls[:, 0] << 4) | (cls[:, 1] << 2) | cls[:, 2]
        order = np.argsort(joint, kind="stable")
        cnt = np.bincount(joint, minlength=64)
        cores.append((sl, order, cnt))
        ncols = np.maximum(ncols, -(-cnt // P))

    ncols_key = tuple(int(x) for x in ncols)
    if ncols_key not in _cache:
        _cache[ncols_key] = _build(ncols_key)
    nc, L = _cache[ncols_key]

    in_maps = []
    entry_pos_all = []
    for sl, order, cnt in cores:
        gstart = np.concatenate([[0], np.cumsum(cnt)])
        n = sl.shape[0]
        # entry -> padded slot position (slot grid: group-major, col*128+p)
        entry_pos = np.empty(n, dtype=np.int64)
        pos_of_sorted = (
            np.repeat(L["gcol"][:-1] * P, cnt)
            + np.arange(n)
            - np.repeat(gstart[:-1], cnt)
        )
        entry_pos[order] = pos_of_sorted
        entry_pos_all.append(entry_pos)

        segs = []
        for pi in range(N_PASS):
            pc = L["pass_cols"][pi]
            for m in range(3):
                stream = np.zeros(pc * P, dtype=np.int16)
                rows_m = sl[order, m]  # slot order
                for g in L["idx_groups"][pi][m]:
                    s0, s1 = gstart[g], gstart[g + 1]
                    dst = int(L["g_tile_off"][pi, m, g]) * P
                    stream[dst : dst + (s1 - s0)] = (rows_m[s0:s1] >> 2).astype(
                        np.int16
                    )
                segs.append(wrap_idxs(stream))
        idx_arr = np.ascontiguousarray(np.concatenate(segs, axis=1))
        in_maps.append(
            {"idx16": idx_arr, "f0p": ftabs[0], "f1p": ftabs[1], "f2p": ftabs[2]}
        )
    return nc, L, in_maps, entry_pos_all


def run(inputs: dict, trace: bool = False):
    from concourse.bass_utils import run_bass_kernel_spmd

    nc, L, in_maps, entry_pos_all = _prep(
        inputs["idxs"], inputs["f0"], inputs["f1"], inputs["f2"]
    )
    res = run_bass_kernel_spmd(
        nc, in_maps, core_ids=list(range(N_CORES)), trace=trace
    )
    outs = []
    for cidx in range(N_CORES):
        flat = res.results[cidx]["out"]  # [P, T] f32; pos = col*128 + p
        vals = np.ascontiguousarray(flat.T).reshape(-1)  # pos-major
        outs.append(vals[entry_pos_all[cidx]])
    return np.concatenate(outs), res


def kernel(**inputs) -> np.ndarray:
    out, _ = run(inputs, trace=False)
    return out


# revision 5
# speedup vs baseline: 1.8263x; 1.0958x over previous
"""CPD reconstruction v5: dma_gather + joint residue-class slot ordering.

Per core (125k entries), all on-device work in one Bass program:

- Host sorts entries by joint class g = (i0%4)*16 + (i1%4)*4 + (i2%4)
  (64 groups), pads each group to whole 128-slot columns; slot = col*128+p.
  Group column counts are maxed over the 8 cores so one SPMD program fits
  all; per-core group contents differ only in the (host-built) idx arrays.
- Processing in PASSES over group ranges.  Per pass, per mode m, per class
  c: the idx stream = concat of groups g in the pass with c_m(g) == c, each
  padded to whole columns; dma_gather calls chunk the stream (<=15872 idx).
  idx value = row >> 2 (< 25000, int16-safe); the mode's table is viewed
  [25000, 128] fp16 and the in-AP is column-shifted by c*32 so each
  256B-strided descriptor reads exactly the wanted 64B row.
- Gathers land in list order = pass-local stream order; the product runs
  per GROUP with host-computed per-mode tile offsets (contiguous slices),
  2 fp16 muls + rank reduce_sum into the slot-ordered f32 output.
- Host maps the padded slot grid back to entry order (free).
"""

import numpy as np

NNZ = 1_000_000
RANK = 32
ROWS = 100_000
N_CORES = 8
N_PER_CORE = NNZ // N_CORES  # 125_000
P = 128
MAXCOLS = 31  # per-call column cap
N_PASS = 2

_cache: dict = {}


_reg_cache: dict = {}


def dma_gather_raw(gp, out_ap, in_ap, idxs_ap, num_idxs, elem_size,
                   elem_step=None, queue_num=0):
    """bass dma_gather minus the 256B elem_size assert (transpose-only)."""
    import concourse.mybir as mybir
    from concourse.bass import exact_div
    import concourse.ap_utils as ap_utils

    assert idxs_ap.dtype == mybir.dt.int16
    assert in_ap.dtype == out_ap.dtype
    if elem_step is None:
        elem_step = elem_size
    assert ap_utils.ap_is_contiguous(in_ap.ap[1:])
    assert ap_utils.ap_is_contiguous(out_ap.ap[1:])
    assert ap_utils.ap_is_contiguous(idxs_ap.ap[1:])
    assert in_ap.ap[-1][1] == elem_size
    assert out_ap.ap[-1][1] == elem_size
    assert in_ap.ap[0][0] == elem_step
    assert out_ap.ap[0][1] * out_ap.ap[1][1] == -(-num_idxs // 128) * 128
    stride_bytes = elem_step * mybir.dt.size(in_ap.dtype)
    stride_bytes_256 = exact_div(stride_bytes, 256)
    assert stride_bytes_256 < 256
    _in_ap = gp.lower_ap_dma(in_ap, for_custom_bir_dma=True)
    _idxs_ap = gp.lower_ap(idxs_ap)
    _out_ap = gp.lower_ap(out_ap)
    return gp.add_instruction(
        mybir.InstDMAGatherAnt(
            name=gp.bass.get_next_instruction_name(),
            ins=[*_in_ap, _idxs_ap, _num_idxs_reg(gp, num_idxs)],
            outs=[_out_ap],
            transpose=False,
            num_idxs=num_idxs,
            elem_size=elem_size,
            stride_bytes_256=stride_bytes_256,
            gen_mode=0,
            single_packet=False,
            queue_num=queue_num,
            sbuf_tokens_per_rank=0,
            sbuf_free_dim_per_rank=0,
            sbuf_free_dim_pad_per_rank=0,
            sbuf_byte_offset=0,
        )
    )


def _num_idxs_reg(gp, num_idxs):
    key = (id(gp.bass), num_idxs)
    if key not in _reg_cache:
        _reg_cache[key] = gp.lower_val_access(gp.to_reg(num_idxs))
    return _reg_cache[key]


def _split_multi_waits(nc, mybir):
    for blk in nc.m.functions[0].blocks:
        new_insts = []
        for inst in blk.instructions:
            si = inst.sync_info
            if si is not None and si.on_wait and len(si.on_wait) > 1:
                extra, keep = list(si.on_wait[:-1]), [si.on_wait[-1]]
                for j, w in enumerate(extra):
                    new_insts.append(
                        mybir.InstEventSemaphore(
                            name=f"{inst.name}-esw{j}",
                            engine=inst.engine,
                            ins=[],
                            outs=[],
                            sync_info=mybir.SyncInfo(on_wait=[w], on_update=[]),
                        )
                    )
                si.on_wait = keep
            new_insts.append(inst)
        blk.instructions = new_insts


QM = np.stack(
    [(np.arange(64) >> 4) & 3, (np.arange(64) >> 2) & 3, np.arange(64) & 3]
)  # [3, 64]: class of group g per mode


def make_layout(ncols: np.ndarray):
    """Shared (all-core) layout from padded per-group column counts.

    Returns dict with:
      T            total slot columns
      gcol[65]     slot-grid column offset of each group
      passes       list of (ga, gb) group ranges
      streams      per pass, per mode: list of (class, [groups], stream_cols)
      g_tile_off   [n_pass][3][64] tile column offset of each group
      calls        per pass: list of (mode, class, tile_col0, tile_col1,
                   idx_seg_off) with idx_seg_off = column offset into that
                   pass+mode idx segment
      pass_cols    per pass: total columns (same for each mode)
      idx_layout   per pass, per mode: ordered [groups] defining idx order
    """
    gcol = np.concatenate([[0], np.cumsum(ncols)]).astype(np.int64)
    T = int(gcol[-1])
    # split into N_PASS group ranges with ~equal columns
    bounds = [0]
    for k in range(1, N_PASS):
        target = T * k // N_PASS
        bounds.append(int(np.searchsorted(gcol, target)))
    bounds.append(64)
    passes = [(bounds[i], bounds[i + 1]) for i in range(N_PASS)]

    g_tile_off = np.full((N_PASS, 3, 64), -1, dtype=np.int64)
    calls = []
    pass_cols = []
    idx_groups = []
    for pi, (ga, gb) in enumerate(passes):
        pcalls = []
        pgroups = [[], [], []]
        for m in range(3):
            off = 0
            for c in range(4):
                groups = [g for g in range(ga, gb) if QM[m, g] == c and ncols[g] > 0]
                if not groups:
                    continue
                c0 = off
                for g in groups:
                    g_tile_off[pi, m, g] = off
                    off += int(ncols[g])
                    pgroups[m].append(g)
                # chunk [c0, off) into calls
                a = c0
                while a < off:
                    b = min(a + MAXCOLS, off)
                    pcalls.append((m, c, a, b))
                    a = b
            if m == 0:
                pc = off
            else:
                assert off == pc
        calls.append(pcalls)
        pass_cols.append(pc)
        idx_groups.append(pgroups)
    return {
        "T": T,
        "gcol": gcol,
        "passes": passes,
        "g_tile_off": g_tile_off,
        "calls": calls,
        "pass_cols": pass_cols,
        "idx_groups": idx_groups,
    }


def _build(ncols_key):
    import concourse.bass as bass
    import concourse.mybir as mybir
    from concourse.tile import TileContext
    from concourse import library_config
    from concourse.library_overlay import lower_extended_insts

    ncols = np.array(ncols_key, dtype=np.int64)
    L = make_layout(ncols)
    T = L["T"]
    maxpc = max(L["pass_cols"])
    S16_per_pass_mode = [pc * 8 for pc in L["pass_cols"]]  # int16 cols per P
    # idx dram layout: per pass, per mode, stream of pc*128 idx wrapped
    tot_s16 = 3 * sum(S16_per_pass_mode)

    nc = bass.Bass(num_swdge_queues=4)
    ftabs = [
        nc.dram_tensor(f"f{m}p", [ROWS // 4, 128], mybir.dt.float16,
                       kind="ExternalInput")
        for m in range(3)
    ]
    idx_d = nc.dram_tensor("idx16", [P, tot_s16], mybir.dt.int16,
                           kind="ExternalInput")
    out_d = nc.dram_tensor("out", [P, T], mybir.dt.float32, kind="ExternalOutput")

    with TileContext(nc) as tc:
        nc.gpsimd.load_library(library_config.attnmlp)
        with (
            tc.tile_pool(name="io", bufs=1) as io_pool,
            tc.tile_pool(name="gat", bufs=1) as gat_pool,
            tc.tile_pool(name="prd", bufs=2) as prd_pool,
        ):
            out_sb = io_pool.tile([P, T], mybir.dt.float32)
            idx_all = io_pool.tile([P, tot_s16], mybir.dt.int16)
            nc.sync.dma_start(out=idx_all[:], in_=idx_d[:])
            g_tiles = []
            for m in range(3):
                gt = gat_pool.tile(
                    [P, maxpc * RANK], mybir.dt.float16, tag=f"g{m}",
                    name=f"g{m}",
                )
                g_tiles.append(gt)
            qrot = 0
            seg_base = 0
            for pi, (ga, gb) in enumerate(L["passes"]):
                pc = L["pass_cols"][pi]
                for (m, c, a, b) in L["calls"][pi]:
                    nidx = (b - a) * P
                    dma_gather_raw(
                        nc.gpsimd,
                        out_ap=g_tiles[m][:, a * RANK : b * RANK].rearrange(
                            "p (s r) -> p s r", r=RANK
                        ),
                        in_ap=ftabs[m][:, c * RANK : (c + 1) * RANK],
                        idxs_ap=idx_all[
                            :,
                            seg_base + m * pc * 8 + a * 8 : seg_base
                            + m * pc * 8
                            + b * 8,
                        ],
                        num_idxs=nidx,
                        elem_size=RANK,
                        elem_step=128,
                        queue_num=qrot % 4,
                    )
                    qrot += 1
                seg_base += 3 * pc * 8
                # products per group
                for g in range(ga, gb):
                    n_g = int(ncols[g])
                    if n_g == 0:
                        continue
                    outc = int(L["gcol"][g])
                    v = [
                        g_tiles[m][
                            :,
                            int(L["g_tile_off"][pi, m, g])
                            * RANK : (int(L["g_tile_off"][pi, m, g]) + n_g)
                            * RANK,
                        ].rearrange("p (s r) -> p s r", r=RANK)
                        for m in range(3)
                    ]
                    tmp = prd_pool.tile([P, 20 * RANK], mybir.dt.float16, tag="t")
                    j0 = 0
                    while j0 < n_g:
                        C = min(20, n_g - j0)
                        tv = tmp[:, : C * RANK].rearrange(
                            "p (s r) -> p s r", r=RANK
                        )
                        nc.vector.tensor_mul(
                            out=tv,
                            in0=v[0][:, j0 : j0 + C],
                            in1=v[1][:, j0 : j0 + C],
                        )
                        nc.vector.tensor_mul(out=tv, in0=tv, in1=v[2][:, j0 : j0 + C])
                        nc.vector.reduce_sum(
                            out=out_sb[:, outc + j0 : outc + j0 + C],
                            in_=tv,
                            axis=mybir.AxisListType.X,
                        )
                        j0 += C
            nc.sync.dma_start(out=out_d[:], in_=out_sb[:])

    _fix_gather_queues(nc, mybir)
    lower_extended_insts(nc)
    _split_multi_waits(nc, mybir)
    return nc, L


def _fix_gather_queues(nc, mybir):
    """Align each gather's SWDGE queue with its Tile-assigned DMASW sem lane
    (a sem lane must only ever be updated from one queue)."""
    import re

    for blk in nc.m.functions[0].blocks:
        for inst in blk.instructions:
            if isinstance(inst, mybir.InstDMAGatherAnt):
                si = inst.sync_info
                lane = None
                for u in list(getattr(si, "on_update", []) or []):
                    m = re.search(r"DMASW(\d+)", repr(u))
                    if m:
                        lane = int(m.group(1))
                        break
                assert lane is not None, f"no DMASW sem on {inst.name}"
                inst.queue_num = lane % 4


def wrap_idxs(idx: np.ndarray) -> np.ndarray:
    """[N] -> [128, N/16] int16: idx i at [i%16, i//16], replicated x8."""
    n = len(idx)
    assert n % 16 == 0
    w = idx.reshape(n // 16, 16).T.astype(np.int16)
    return np.tile(w, (8, 1))


def _prep(idxs, f0, f1, f2):
    idxs = np.asarray(idxs)
    idx3_full = idxs.astype(np.int32) if idxs.dtype != np.int32 else idxs
    ftabs = [
        np.ascontiguousarray(
            np.asarray(f, dtype=np.float32).astype(np.float16).reshape(ROWS // 4, 128)
        )
        for f in (f0, f1, f2)
    ]

    cores = []
    ncols = np.zeros(64, dtype=np.int64)
    for cidx in range(N_CORES):
        sl = idx3_full[cidx * N_PER_CORE : (cidx + 1) * N_PER_CORE]
        cls = sl & 3
        joint = (cls[:, 0] << 4) | (cls[:, 1] << 2) | cls[:, 2]
        order = np.argsort(joint, kind="stable")
        cnt = np.bincount(joint, minlength=64)
        cores.append((sl, order, cnt))
        ncols = np.maximum(ncols, -(-cnt // P))

    ncols_key = tuple(int(x) for x in ncols)
    if ncols_key not in _cache:
        _cache[ncols_key] = _build(ncols_key)
    nc, L = _cache[ncols_key]

    in_maps = []
    entry_pos_all = []
    for sl, order, cnt in cores:
        gstart = np.concatenate([[0], np.cumsum(cnt)])
        n = sl.shape[0]
        # entry -> padded slot position (slot grid: group-major, col*128+p)
        entry_pos = np.empty(n, dtype=np.int64)
        pos_of_sorted = (
            np.repeat(L["gcol"][:-1] * P, cnt)
            + np.arange(n)
            - np.repeat(gstart[:-1], cnt)
        )
        entry_pos[order] = pos_of_sorted
        entry_pos_all.append(entry_pos)

        segs = []
        for pi in range(N_PASS):
            pc = L["pass_cols"][pi]
            for m in range(3):
                stream = np.zeros(pc * P, dtype=np.int16)
                rows_m = sl[order, m]  # slot order
                for g in L["idx_groups"][pi][m]:
                    s0, s1 = gstart[g], gstart[g + 1]
                    dst = int(L["g_tile_off"][pi, m, g]) * P
                    stream[dst : dst + (s1 - s0)] = (rows_m[s0:s1] >> 2).astype(
                        np.int16
                    )
                segs.append(wrap_idxs(stream))
        idx_arr = np.ascontiguousarray(np.concatenate(segs, axis=1))
        in_maps.append(
            {"idx16": idx_arr, "f0p": ftabs[0], "f1p": ftabs[1], "f2p": ftabs[2]}
        )
    return nc, L, in_maps, entry_pos_all


def run(inputs: dict, trace: bool = False):
    from concourse.bass_utils import run_bass_kernel_spmd

    nc, L, in_maps, entry_pos_all = _prep(
        inputs["idxs"], inputs["f0"], inputs["f1"], inputs["f2"]
    )
    res = run_bass_kernel_spmd(
        nc, in_maps, core_ids=list(range(N_CORES)), trace=trace
    )
    outs = []
    for cidx in range(N_CORES):
        flat = res.results[cidx]["out"]  # [P, T] f32; pos = col*128 + p
        vals = np.ascontiguousarray(flat.T).reshape(-1)  # pos-major
        outs.append(vals[entry_pos_all[cidx]])
    return np.concatenate(outs), res


def kernel(**inputs) -> np.ndarray:
    out, _ = run(inputs, trace=False)
    return out


# revision 6
# speedup vs baseline: 1.9636x; 1.0751x over previous
"""CPD reconstruction v5: dma_gather + joint residue-class slot ordering.

Per core (125k entries), all on-device work in one Bass program:

- Host sorts entries by joint class g = (i0%4)*16 + (i1%4)*4 + (i2%4)
  (64 groups), pads each group to whole 128-slot columns; slot = col*128+p.
  Group column counts are maxed over the 8 cores so one SPMD program fits
  all; per-core group contents differ only in the (host-built) idx arrays.
- Processing in PASSES over group ranges.  Per pass, per mode m, per class
  c: the idx stream = concat of groups g in the pass with c_m(g) == c, each
  padded to whole columns; dma_gather calls chunk the stream (<=15872 idx).
  idx value = row >> 2 (< 25000, int16-safe); the mode's table is viewed
  [25000, 128] fp16 and the in-AP is column-shifted by c*32 so each
  256B-strided descriptor reads exactly the wanted 64B row.
- Gathers land in list order = pass-local stream order; the product runs
  per GROUP with host-computed per-mode tile offsets (contiguous slices),
  2 fp16 muls + rank reduce_sum into the slot-ordered f32 output.
- Host maps the padded slot grid back to entry order (free).
"""

import numpy as np

NNZ = 1_000_000
RANK = 32
ROWS = 100_000
N_CORES = 8
N_PER_CORE = NNZ // N_CORES  # 125_000
P = 128
MAXCOLS = 16  # per-call column cap
N_PASS = 2

_cache: dict = {}


_reg_cache: dict = {}


def dma_gather_raw(gp, out_ap, in_ap, idxs_ap, num_idxs, elem_size,
                   elem_step=None, queue_num=0):
    """bass dma_gather minus the 256B elem_size assert (transpose-only)."""
    import concourse.mybir as mybir
    from concourse.bass import exact_div
    import concourse.ap_utils as ap_utils

    assert idxs_ap.dtype == mybir.dt.int16
    assert in_ap.dtype == out_ap.dtype
    if elem_step is None:
        elem_step = elem_size
    assert ap_utils.ap_is_contiguous(in_ap.ap[1:])
    assert ap_utils.ap_is_contiguous(out_ap.ap[1:])
    assert ap_utils.ap_is_contiguous(idxs_ap.ap[1:])
    assert in_ap.ap[-1][1] == elem_size
    assert out_ap.ap[-1][1] == elem_size
    assert in_ap.ap[0][0] == elem_step
    assert out_ap.ap[0][1] * out_ap.ap[1][1] == -(-num_idxs // 128) * 128
    stride_bytes = elem_step * mybir.dt.size(in_ap.dtype)
    stride_bytes_256 = exact_div(stride_bytes, 256)
    assert stride_bytes_256 < 256
    _in_ap = gp.lower_ap_dma(in_ap, for_custom_bir_dma=True)
    _idxs_ap = gp.lower_ap(idxs_ap)
    _out_ap = gp.lower_ap(out_ap)
    return gp.add_instruction(
        mybir.InstDMAGatherAnt(
            name=gp.bass.get_next_instruction_name(),
            ins=[*_in_ap, _idxs_ap, _num_idxs_reg(gp, num_idxs)],
            outs=[_out_ap],
            transpose=False,
            num_idxs=num_idxs,
            elem_size=elem_size,
            stride_bytes_256=stride_bytes_256,
            gen_mode=0,
            single_packet=False,
            queue_num=queue_num,
            sbuf_tokens_per_rank=0,
            sbuf_free_dim_per_rank=0,
            sbuf_free_dim_pad_per_rank=0,
            sbuf_byte_offset=0,
        )
    )


def _num_idxs_reg(gp, num_idxs):
    key = (id(gp.bass), num_idxs)
    if key not in _reg_cache:
        _reg_cache[key] = gp.lower_val_access(gp.to_reg(num_idxs))
    return _reg_cache[key]


def _split_multi_waits(nc, mybir):
    for blk in nc.m.functions[0].blocks:
        new_insts = []
        for inst in blk.instructions:
            si = inst.sync_info
            if si is not None and si.on_wait and len(si.on_wait) > 1:
                extra, keep = list(si.on_wait[:-1]), [si.on_wait[-1]]
                for j, w in enumerate(extra):
                    new_insts.append(
                        mybir.InstEventSemaphore(
                            name=f"{inst.name}-esw{j}",
                            engine=inst.engine,
                            ins=[],
                            outs=[],
                            sync_info=mybir.SyncInfo(on_wait=[w], on_update=[]),
                        )
                    )
                si.on_wait = keep
            new_insts.append(inst)
        blk.instructions = new_insts


QM = np.stack(
    [(np.arange(64) >> 4) & 3, (np.arange(64) >> 2) & 3, np.arange(64) & 3]
)  # [3, 64]: class of group g per mode


def make_layout(ncols: np.ndarray):
    """Shared (all-core) layout from padded per-group column counts.

    Returns dict with:
      T            total slot columns
      gcol[65]     slot-grid column offset of each group
      passes       list of (ga, gb) group ranges
      streams      per pass, per mode: list of (class, [groups], stream_cols)
      g_tile_off   [n_pass][3][64] tile column offset of each group
      calls        per pass: list of (mode, class, tile_col0, tile_col1,
                   idx_seg_off) with idx_seg_off = column offset into that
                   pass+mode idx segment
      pass_cols    per pass: total columns (same for each mode)
      idx_layout   per pass, per mode: ordered [groups] defining idx order
    """
    gcol = np.concatenate([[0], np.cumsum(ncols)]).astype(np.int64)
    T = int(gcol[-1])
    # split into N_PASS group ranges with ~equal columns
    bounds = [0]
    for k in range(1, N_PASS):
        target = T * k // N_PASS
        bounds.append(int(np.searchsorted(gcol, target)))
    bounds.append(64)
    passes = [(bounds[i], bounds[i + 1]) for i in range(N_PASS)]

    g_tile_off = np.full((N_PASS, 3, 64), -1, dtype=np.int64)
    calls = []
    pass_cols = []
    idx_groups = []
    for pi, (ga, gb) in enumerate(passes):
        pcalls = []
        pgroups = [[], [], []]
        for m in range(3):
            off = 0
            for c in range(4):
                groups = [g for g in range(ga, gb) if QM[m, g] == c and ncols[g] > 0]
                if not groups:
                    continue
                c0 = off
                for g in groups:
                    g_tile_off[pi, m, g] = off
                    off += int(ncols[g])
                    pgroups[m].append(g)
                # chunk [c0, off) into calls
                a = c0
                while a < off:
                    b = min(a + MAXCOLS, off)
                    pcalls.append((m, c, a, b))
                    a = b
            if m == 0:
                pc = off
            else:
                assert off == pc
        calls.append(pcalls)
        pass_cols.append(pc)
        idx_groups.append(pgroups)
    return {
        "T": T,
        "gcol": gcol,
        "passes": passes,
        "g_tile_off": g_tile_off,
        "calls": calls,
        "pass_cols": pass_cols,
        "idx_groups": idx_groups,
    }


def _build(ncols_key):
    import concourse.bass as bass
    import concourse.mybir as mybir
    from concourse.tile import TileContext
    from concourse import library_config
    from concourse.library_overlay import lower_extended_insts

    ncols = np.array(ncols_key, dtype=np.int64)
    L = make_layout(ncols)
    T = L["T"]
    maxpc = max(L["pass_cols"])
    S16_per_pass_mode = [pc * 8 for pc in L["pass_cols"]]  # int16 cols per P
    # idx dram layout: per pass, per mode, stream of pc*128 idx wrapped
    tot_s16 = 3 * sum(S16_per_pass_mode)

    nc = bass.Bass(num_swdge_queues=4)
    ftabs = [
        nc.dram_tensor(f"f{m}p", [ROWS // 4, 128], mybir.dt.float16,
                       kind="ExternalInput")
        for m in range(3)
    ]
    idx_d = nc.dram_tensor("idx16", [P, tot_s16], mybir.dt.int16,
                           kind="ExternalInput")
    out_d = nc.dram_tensor("out", [P, T], mybir.dt.float32, kind="ExternalOutput")

    with TileContext(nc) as tc:
        nc.gpsimd.load_library(library_config.attnmlp)
        with (
            tc.tile_pool(name="io", bufs=1) as io_pool,
            tc.tile_pool(name="gat", bufs=1) as gat_pool,
            tc.tile_pool(name="prd", bufs=2) as prd_pool,
        ):
            out_sb = io_pool.tile([P, T], mybir.dt.float32)
            idx_all = io_pool.tile([P, tot_s16], mybir.dt.int16)
            nc.sync.dma_start(out=idx_all[:], in_=idx_d[:])
            g_tiles = []
            for m in range(3):
                gt = gat_pool.tile(
                    [P, maxpc * RANK], mybir.dt.float16, tag=f"g{m}",
                    name=f"g{m}",
                )
                g_tiles.append(gt)
            qrot = 0
            seg_base = 0
            for pi, (ga, gb) in enumerate(L["passes"]):
                pc = L["pass_cols"][pi]
                for (m, c, a, b) in L["calls"][pi]:
                    nidx = (b - a) * P
                    dma_gather_raw(
                        nc.gpsimd,
                        out_ap=g_tiles[m][:, a * RANK : b * RANK].rearrange(
                            "p (s r) -> p s r", r=RANK
                        ),
                        in_ap=ftabs[m][:, c * RANK : (c + 1) * RANK],
                        idxs_ap=idx_all[
                            :,
                            seg_base + m * pc * 8 + a * 8 : seg_base
                            + m * pc * 8
                            + b * 8,
                        ],
                        num_idxs=nidx,
                        elem_size=RANK,
                        elem_step=128,
                        queue_num=qrot % 4,
                    )
                    qrot += 1
                seg_base += 3 * pc * 8
                # products per group
                for g in range(ga, gb):
                    n_g = int(ncols[g])
                    if n_g == 0:
                        continue
                    outc = int(L["gcol"][g])
                    v = [
                        g_tiles[m][
                            :,
                            int(L["g_tile_off"][pi, m, g])
                            * RANK : (int(L["g_tile_off"][pi, m, g]) + n_g)
                            * RANK,
                        ].rearrange("p (s r) -> p s r", r=RANK)
                        for m in range(3)
                    ]
                    tmp = prd_pool.tile([P, 20 * RANK], mybir.dt.float16, tag="t")
                    j0 = 0
                    while j0 < n_g:
                        C = min(20, n_g - j0)
                        tv = tmp[:, : C * RANK].rearrange(
                            "p (s r) -> p s r", r=RANK
                        )
                        nc.vector.tensor_mul(
                            out=tv,
                            in0=v[0][:, j0 : j0 + C],
                            in1=v[1][:, j0 : j0 + C],
                        )
                        nc.vector.tensor_mul(out=tv, in0=tv, in1=v[2][:, j0 : j0 + C])
                        nc.vector.reduce_sum(
                            out=out_sb[:, outc + j0 : outc + j0 + C],
                            in_=tv,
                            axis=mybir.AxisListType.X,
                        )
                        j0 += C
            nc.sync.dma_start(out=out_d[:], in_=out_sb[:])

    _fix_gather_queues(nc, mybir)
    lower_extended_insts(nc)
    _split_multi_waits(nc, mybir)
    return nc, L


def _fix_gather_queues(nc, mybir):
    """Align each gather's SWDGE queue with its Tile-assigned DMASW sem lane
    (a sem lane must only ever be updated from one queue)."""
    import re

    for blk in nc.m.functions[0].blocks:
        for inst in blk.instructions:
            if isinstance(inst, mybir.InstDMAGatherAnt):
                si = inst.sync_info
                lane = None
                for u in list(getattr(si, "on_update", []) or []):
                    m = re.search(r"DMASW(\d+)", repr(u))
                    if m:
                        lane = int(m.group(1))
                        break
                assert lane is not None, f"no DMASW sem on {inst.name}"
                inst.queue_num = lane % 4


def wrap_idxs(idx: np.ndarray) -> np.ndarray:
    """[N] -> [128, N/16] int16: idx i at [i%16, i//16], replicated x8."""
    n = len(idx)
    assert n % 16 == 0
    w = idx.reshape(n // 16, 16).T.astype(np.int16)
    return np.tile(w, (8, 1))


def _prep(idxs, f0, f1, f2):
    idxs = np.asarray(idxs)
    idx3_full = idxs.astype(np.int32) if idxs.dtype != np.int32 else idxs
    ftabs = [
        np.ascontiguousarray(
            np.asarray(f, dtype=np.float32).astype(np.float16).reshape(ROWS // 4, 128)
        )
        for f in (f0, f1, f2)
    ]

    cores = []
    ncols = np.zeros(64, dtype=np.int64)
    for cidx in range(N_CORES):
        sl = idx3_full[cidx * N_PER_CORE : (cidx + 1) * N_PER_CORE]
        cls = sl & 3
        joint = (cls[:, 0] << 4) | (cls[:, 1] << 2) | cls[:, 2]
        order = np.argsort(joint, kind="stable")
        cnt = np.bincount(joint, minlength=64)
        cores.append((sl, order, cnt))
        ncols = np.maximum(ncols, -(-cnt // P))

    ncols_key = tuple(int(x) for x in ncols)
    if ncols_key not in _cache:
        _cache[ncols_key] = _build(ncols_key)
    nc, L = _cache[ncols_key]

    in_maps = []
    entry_pos_all = []
    for sl, order, cnt in cores:
        gstart = np.concatenate([[0], np.cumsum(cnt)])
        n = sl.shape[0]
        # entry -> padded slot position (slot grid: group-major, col*128+p)
        entry_pos = np.empty(n, dtype=np.int64)
        pos_of_sorted = (
            np.repeat(L["gcol"][:-1] * P, cnt)
            + np.arange(n)
            - np.repeat(gstart[:-1], cnt)
        )
        entry_pos[order] = pos_of_sorted
        entry_pos_all.append(entry_pos)

        segs = []
        for pi in range(N_PASS):
            pc = L["pass_cols"][pi]
            for m in range(3):
                stream = np.zeros(pc * P, dtype=np.int16)
                rows_m = sl[order, m]  # slot order
                for g in L["idx_groups"][pi][m]:
                    s0, s1 = gstart[g], gstart[g + 1]
                    dst = int(L["g_tile_off"][pi, m, g]) * P
                    stream[dst : dst + (s1 - s0)] = (rows_m[s0:s1] >> 2).astype(
                        np.int16
                    )
                segs.append(wrap_idxs(stream))
        idx_arr = np.ascontiguousarray(np.concatenate(segs, axis=1))
        in_maps.append(
            {"idx16": idx_arr, "f0p": ftabs[0], "f1p": ftabs[1], "f2p": ftabs[2]}
        )
    return nc, L, in_maps, entry_pos_all


def run(inputs: dict, trace: bool = False):
    from concourse.bass_utils import run_bass_kernel_spmd

    nc, L, in_maps, entry_pos_all = _prep(
        inputs["idxs"], inputs["f0"], inputs["f1"], inputs["f2"]
    )
    res = run_bass_kernel_spmd(
        nc, in_maps, core_ids=list(range(N_CORES)), trace=trace
    )
    outs = []
    for cidx in range(N_CORES):
        flat = res.results[cidx]["out"]  # [P, T] f32; pos = col*128 + p
        vals = np.ascontiguousarray(flat.T).reshape(-1)  # pos-major
        outs.append(vals[entry_pos_all[cidx]])
    return np.concatenate(outs), res


def kernel(**inputs) -> np.ndarray:
    out, _ = run(inputs, trace=False)
    return out
